# revision 4
# baseline (speedup 1.0000x reference)
"""Multi-head attention (B=16, S=1024, D=768, H=12) on 8 TRN2 NeuronCores.

Strategy: pure data parallelism — batch 16 is split 2-per-core; weights are
replicated. Each core runs an identical Bass/Tile program on its own x shard,
so no collectives are needed. Host-side marshaling pre-transposes x and the
weights into the d-major layouts the PE array contracts over.

Per-core program (b in 0..1, head-pairs hp in 0..5):
  - v  = x @ W_v^T           natural [t, e] layout, stored head-interleaved
                             with a ones column -> PV lhsT [k, 64+1] per head
  - qT2/kT2 [128, S]         two heads stacked on partitions (d-major)
  - scoresT[k,q] = k q^T     row-packed per head via tile_position (K=64)
  - exp on ACT (scale=1/8) -> f32r SBUF tile
  - PV: out[dh+1, q] += v_ext.T @ exp   (row 64 accumulates the softmax denom)
  - 1/denom on DVE; K=1 ones-matmul broadcasts it across 64 partitions;
    DVE mult normalizes into attn_outT [d, t]
  - y = attn_outT.T @ W_out^T + b_out  (bias folded in as a K=1 matmul)

All matmuls run as float32r (1 cycle/row at free-dim >=256, ~1.5e-4 rel err).
"""
import numpy as np
import concourse.bacc as bacc
import concourse.tile as tile
from concourse import mybir
from concourse.bass_utils import run_bass_kernel_spmd

FP32 = mybir.dt.float32
FP32R = mybir.dt.float32r
EXP = mybir.ActivationFunctionType.Exp

B, S, D, H = 2, 1024, 768, 12       # per-core batch of 2
HP = H // 2                          # head pairs
DT = D // 128                        # d tiles (6)
KT = S // 128                        # k tiles (8)
QC = S // 512                        # q chunks (2)
TT = S // 128                        # t tiles per batch (8)
N_CORES = 8

_CACHE = {}


def build_nc():
    nc = bacc.Bacc(trn_type="TRN2")
    xT = nc.dram_tensor("xT", [D, B * S], FP32R, kind="ExternalInput")
    wqkvT = nc.dram_tensor("wqkvT", [D, 3 * D], FP32R, kind="ExternalInput")
    woutT = nc.dram_tensor("woutT", [D, D], FP32R, kind="ExternalInput")
    bout = nc.dram_tensor("bout", [1, D], FP32R, kind="ExternalInput")
    ones_d = nc.dram_tensor("ones_d", [128, 128], FP32R, kind="ExternalInput")
    y = nc.dram_tensor("y", [B * S, D], FP32, kind="ExternalOutput")

    with tile.TileContext(nc) as tc:
        with (
            tc.tile_pool(name="wq", bufs=1) as p_wq,
            tc.tile_pool(name="wo", bufs=1) as p_wo,
            tc.tile_pool(name="cst", bufs=1) as p_cst,
            tc.tile_pool(name="xt", bufs=1) as p_xt,
            tc.tile_pool(name="vv", bufs=1) as p_v,
            tc.tile_pool(name="ao", bufs=1) as p_ao,
            tc.tile_pool(name="qk", bufs=4) as p_qk,
            tc.tile_pool(name="exp", bufs=3) as p_exp,
            tc.tile_pool(name="rr", bufs=2) as p_r,
            tc.tile_pool(name="yy", bufs=2) as p_y,
            tc.tile_pool(name="rb", bufs=2) as p_rb,
            tc.tile_pool(name="mm", bufs=2, space="PSUM") as p_mm,
            tc.tile_pool(name="oacc", bufs=4, space="PSUM") as p_oacc,
        ):
            wq = p_wq.tile([128, DT, 3 * D], FP32R)
            wo = p_wo.tile([128, DT, D], FP32R)
            for j in range(DT):
                nc.sync.dma_start(wq[:, j, :], wqkvT[128 * j:128 * (j + 1), :])
                nc.sync.dma_start(wo[:, j, :], woutT[128 * j:128 * (j + 1), :])
            bo = p_cst.tile([1, D], FP32R)
            nc.sync.dma_start(bo[:], bout[:])
            ones = p_cst.tile([1, 128], FP32R)
            nc.sync.dma_start(ones[:], ones_d[0:1, :])

            for b in range(B):
                xt = p_xt.tile([128, DT, S], FP32R, tag="xt")
                for j in range(DT):
                    nc.sync.dma_start(
                        xt[:, j, :], xT[128 * j:128 * (j + 1), b * S:(b + 1) * S]
                    )

                # ---- v generation: v[t, e] for all 12 heads, head-interleaved
                # [128, kt, h, 65] with col 64 = 1.0 (softmax denom rider)
                v = p_v.tile([128, KT, H, 65], FP32R, tag="vv")
                nc.sync.dma_start(
                    v[:, :, :, 64],
                    ones_d[:, 0:KT * H].rearrange("p (k h) -> p k h", k=KT),
                )
                for tt in range(TT):
                    vp = p_mm.tile([128, 1024], FP32, tag="mm")
                    for c0, cw in ((0, 512), (512, 256)):
                        for j in range(DT):
                            nc.tensor.matmul(
                                vp[:, c0:c0 + cw],
                                xt[:, j, tt * 128:(tt + 1) * 128],
                                wq[:, j, 2 * D + c0:2 * D + c0 + cw],
                                start=(j == 0), stop=(j == DT - 1),
                            )
                    nc.vector.tensor_copy(
                        v[:, tt, :, 0:64],
                        vp[:, 0:768].rearrange("p (h c) -> p h c", h=H),
                    )

                # attn_outT [d, t] accumulator for this batch; each hp writes
                # a disjoint d-tile band
                ao = p_ao.tile([128, DT, S], FP32R, tag="ao")

                for hp in range(HP):
                    # ---- q/k generation for this head pair (2 heads stacked)
                    qkt = []
                    for part in range(2):  # 0 = q, 1 = k
                        qp = p_mm.tile([128, 1024], FP32, tag="mm")
                        for qc in range(QC):
                            for j in range(DT):
                                nc.tensor.matmul(
                                    qp[:, qc * 512:(qc + 1) * 512],
                                    wq[:, j,
                                       part * D + 128 * hp:part * D + 128 * (hp + 1)],
                                    xt[:, j, qc * 512:(qc + 1) * 512],
                                    start=(j == 0), stop=(j == DT - 1),
                                )
                        sq = p_qk.tile([128, S], FP32R, tag="qk")
                        nc.vector.tensor_copy(sq[:], qp[:])
                        qkt.append(sq)
                    qT2, kT2 = qkt

                    for qc in range(QC):
                        oA = p_oacc.tile([65, 512], FP32, tag="oacc")
                        oB = p_oacc.tile([65, 512], FP32, tag="oacc")
                        for kt in range(KT):
                            sc = p_mm.tile([128, 1024], FP32, tag="mm")
                            nc.tensor.matmul(
                                sc[:, 0:512],
                                kT2[0:64, kt * 128:(kt + 1) * 128],
                                qT2[0:64, qc * 512:(qc + 1) * 512],
                                start=True, stop=True, tile_position=(0, 0),
                            )
                            nc.tensor.matmul(
                                sc[:, 512:1024],
                                kT2[64:128, kt * 128:(kt + 1) * 128],
                                qT2[64:128, qc * 512:(qc + 1) * 512],
                                start=True, stop=True, tile_position=(64, 0),
                            )
                            ex = p_exp.tile([128, 1024], FP32R, tag="exp")
                            nc.scalar.activation(ex[:], sc[:], EXP, scale=0.125)
                            nc.tensor.matmul(
                                oA[:], v[:, kt, 2 * hp, :], ex[:, 0:512],
                                start=(kt == 0), stop=(kt == KT - 1),
                            )
                            nc.tensor.matmul(
                                oB[:], v[:, kt, 2 * hp + 1, :], ex[:, 512:1024],
                                start=(kt == 0), stop=(kt == KT - 1),
                            )
                        # normalize: recip of denom row, gpsimd broadcast
                        # across partitions, DVE mult into attn_outT (head B
                        # shifts partitions on the write)
                        for o, head in ((oA, 0), (oB, 1)):
                            r = p_r.tile([1, 512], FP32, tag="rr")
                            nc.vector.reciprocal(r[:], o[64:65, :])
                            rb = p_rb.tile([64, 512], FP32, tag="rb")
                            nc.gpsimd.partition_broadcast(rb[:], r[:])
                            nc.vector.tensor_mul(
                                ao[64 * head:64 * (head + 1), hp,
                                   qc * 512:(qc + 1) * 512],
                                o[0:64, :], rb[:],
                            )

                # ---- output projection for this batch
                for tt in range(TT):
                    yp = p_mm.tile([128, 1024], FP32, tag="mm")
                    for c0, cw in ((0, 512), (512, 256)):
                        nc.tensor.matmul(
                            yp[:, c0:c0 + cw], ones[:], bo[:, c0:c0 + cw],
                            start=True, stop=False,
                        )
                        for j in range(DT):
                            nc.tensor.matmul(
                                yp[:, c0:c0 + cw],
                                ao[:, j, tt * 128:(tt + 1) * 128],
                                wo[:, j, c0:c0 + cw],
                                start=False, stop=(j == DT - 1),
                            )
                    ys = p_y.tile([128, D], FP32, tag="yy")
                    nc.vector.tensor_copy(ys[:], yp[:, 0:768])
                    nc.sync.dma_start(
                        y[b * S + tt * 128:b * S + (tt + 1) * 128, :], ys[:]
                    )
    nc.finalize()
    return nc


def _marshal(x, W_qkv, W_out, b_out):
    wqkvT = np.ascontiguousarray(W_qkv.T).astype(np.float32)
    woutT = np.ascontiguousarray(W_out.T).astype(np.float32)
    bo = np.ascontiguousarray(b_out.reshape(1, D)).astype(np.float32)
    ones = np.ones((128, 128), dtype=np.float32)
    in_maps = []
    for c in range(N_CORES):
        xc = np.ascontiguousarray(
            np.asarray(x)[B * c:B * (c + 1)].reshape(B * S, D).T
        ).astype(np.float32)
        in_maps.append({
            "xT": xc, "wqkvT": wqkvT, "woutT": woutT, "bout": bo,
            "ones_d": ones,
        })
    return in_maps


def run(x, W_qkv, W_out, b_out, trace=False, **spmd_kwargs):
    if "nc" not in _CACHE:
        _CACHE["nc"] = build_nc()
    nc = _CACHE["nc"]
    in_maps = _marshal(x, W_qkv, W_out, b_out)
    res = run_bass_kernel_spmd(
        nc, in_maps, core_ids=list(range(N_CORES)), trace=trace, **spmd_kwargs
    )
    out = np.stack([res.results[c]["y"] for c in range(N_CORES)], axis=0)
    out = out.reshape(N_CORES * B, S, D)
    return out, res


def kernel(x, W_qkv, W_out, b_out):
    out, _ = run(x, W_qkv, W_out, b_out)
    return out
